# revision 46
# baseline (speedup 1.0000x reference)
"""KNN retrieval kernel for Trainium2 (8 NeuronCores, SPMD).

Pipeline (v3 — fp8 DoubleRow matmul + device window-max, host top-k):
  host:   L2-normalize datastore rows, scale x16, quantize to fp8 e4m3,
          shard N=500000 -> 8 x 62500, transpose to [512, N_loc] per core.
          Queries quantized to fp8 e4m3 (per-query scale is irrelevant to
          per-query ranking).
  device: sims = qT.T @ fT in fp8 DoubleRow (2 fp8 MACs/cell, fp32 PSUM).
          Per 512-col PSUM bank: window-max over 32-col windows
          (vector.tensor_reduce, one fused pass per 4-bank chunk) -> fp16
          SBUF.  That is the whole device reduction; the window-max array
          [2, 128, 1984] per core (1 MB) streams out in two DMA halves.
  host:   per query: take top-96 windows by device value across all cores
          (8*1984 = 15872 candidates), exactly rescore all 32 columns of
          each (fp32 prescore + fp64 refine, true L2 normalization), exact
          top-32 with the reference tie-break, W = exp(sim/T), one-hot
          label aggregation.

Recall: a true top-32 neighbor is missed only if its 32-col window is not
in the query's global top-96 windows by device value (measured max true-
item window rank 52 under fp8 quantization noise of ~2.5e-3 cosine units).
In-window collisions are harmless: the host rescores whole windows.
"""

import numpy as np
import ml_dtypes

import concourse.bass as bass
import concourse.mybir as mybir
from concourse import bacc
from concourse.tile import TileContext
from concourse import bass_utils

P = 128
D = 512            # feature dim (4 K-subtiles of 128; DoubleRow pairs 2)
KC = D // P        # 4
NQ = 256           # queries (2 partition blocks)
QB = NQ // P       # 2
NCORES = 8
N_TOTAL = 500000
N_SHARD = N_TOTAL // NCORES   # 62500
CB = 2048                      # DMA column block (one 4-bank PSUM tile and
                               # one fused window-max pass per query block)
NCOLB = (N_SHARD + CB - 1) // CB   # 31
N_PAD = CB * NCOLB                 # 63488
BANK = 512                         # one PSUM bank of fp32
WIN = 32                           # window-max width
NW_BANK = BANK // WIN              # 16
NWIN = (N_PAD - BANK) // WIN       # 1968: the last chunk's 4th bank is
                                   # pure padding and is never computed

K = 32
TEMP = 0.07
NUM_CLASSES = 1000
EPS = 1e-12
FSCALE = 16.0      # normalized features scaled into fp8's sweet spot
T_WINDOWS = 96     # windows exactly rescored per query

_NC_CACHE = None


def build_nc():
    """Single-core Bass program (run SPMD on all 8 cores)."""
    nc = bacc.Bacc("TRN2", target_bir_lowering=False, debug=False)
    q_dram = nc.dram_tensor("qT", [P, KC, NQ], mybir.dt.float8e4,
                            kind="ExternalInput").ap()
    f_dram = nc.dram_tensor("fT", [D, N_PAD], mybir.dt.float8e4,
                            kind="ExternalInput").ap()
    wm_dram = nc.dram_tensor("wmax", [QB, P, NWIN], mybir.dt.float16,
                             kind="ExternalOutput").ap()

    DR = mybir.MatmulPerfMode.DoubleRow

    with TileContext(nc) as tc:
        with (
            tc.tile_pool(name="qpool", bufs=1) as qpool,
            tc.tile_pool(name="fpool", bufs=6) as fpool,
            tc.tile_pool(name="cpool", bufs=1) as cpool,
            tc.tile_pool(name="psum", bufs=1, space="PSUM") as psum_pool,
        ):
            qt = qpool.tile([P, KC, NQ], mybir.dt.float8e4)
            nc.sync.dma_start(qt, q_dram)

            # PE warm-up: dummy matmuls on the query tile while the first
            # feature chunk is still in flight, so the tensor engine exits
            # its slow p-state before real work begins.  Results land in a
            # PSUM bank that the first real start=True matmul resets.
            wp = psum_pool.tile([P, CB // WIN, WIN], mybir.dt.float32,
                                name="pt0", tag="pt0")
            for _ in range(10):
                nc.tensor.matmul(wp[:, 0:4, :],
                                 lhsT=qt[:, 0:2, 0:P],
                                 rhs=qt[:, 2:4, 0:P],
                                 start=True, stop=True, perf_mode=DR)

            # window-max buffers split three ways: the first two stream out
            # to HBM mid-kernel so only the last two chunks' windows (112)
            # remain for the final DMA after the last reduce
            CB_SPLIT = 20
            CB_SPLIT2 = 29
            W_SPLIT = CB_SPLIT * (CB // WIN)            # 1280
            W_SPLIT2 = CB_SPLIT2 * (CB // WIN)          # 1856
            wma = [cpool.tile([P, W_SPLIT], mybir.dt.float16,
                              name=f"wma{qb}") for qb in range(QB)]
            wmb = [cpool.tile([P, W_SPLIT2 - W_SPLIT], mybir.dt.float16,
                              name=f"wmb{qb}") for qb in range(QB)]
            wmc = [cpool.tile([P, NWIN - W_SPLIT2], mybir.dt.float16,
                              name=f"wmc{qb}") for qb in range(QB)]

            for cb in range(NCOLB):
                ft = fpool.tile([P, KC, CB], mybir.dt.float8e4, tag="ft")
                if cb == 0:
                    # split the first chunk's loads into column halves,
                    # h-major: all four k-subtiles' first halves go to the
                    # early-starting DMA queues, so banks 0-1 can be fully
                    # computed and reduced while the second half lands
                    for h in range(2):
                        for k in range(KC):
                            hw = CB // 2
                            nc.sync.dma_start(
                                ft[:, k, h * hw:(h + 1) * hw],
                                f_dram[k * P:(k + 1) * P,
                                       h * hw:(h + 1) * hw])
                else:
                    for k in range(KC):
                        nc.sync.dma_start(
                            ft[:, k, :],
                            f_dram[k * P:(k + 1) * P, cb * CB:(cb + 1) * CB])

                # the last chunk's 4th bank (cols >= 62976) is all padding
                nbanks = 3 if cb == NCOLB - 1 else CB // BANK
                for qb in range(QB):
                    # one 4-bank PSUM tile per query block; each matmul
                    # writes one bank, then a single fused window-max pass
                    # drains them all (fewer DVE instructions + handoffs)
                    pt = psum_pool.tile([P, CB // WIN, WIN],
                                        mybir.dt.float32,
                                        name=f"pt{qb}", tag=f"pt{qb}")
                    for nn in range(nbanks):
                        for p in range(2):
                            nc.tensor.matmul(
                                pt[:, nn * NW_BANK:(nn + 1) * NW_BANK, :],
                                lhsT=qt[:, 2 * p:2 * p + 2,
                                        qb * P:(qb + 1) * P],
                                rhs=ft[:, 2 * p:2 * p + 2,
                                       nn * BANK:(nn + 1) * BANK],
                                start=(p == 0), stop=(p == 1),
                                perf_mode=DR)
                    nw = CB // WIN
                    nw_cb = nbanks * NW_BANK
                    if cb < CB_SPLIT:
                        dst = wma[qb][:, cb * nw:cb * nw + nw_cb]
                    elif cb < CB_SPLIT2:
                        dst = wmb[qb][:, (cb - CB_SPLIT) * nw:
                                      (cb - CB_SPLIT) * nw + nw_cb]
                    else:
                        dst = wmc[qb][:, (cb - CB_SPLIT2) * nw:
                                      (cb - CB_SPLIT2) * nw + nw_cb]
                    if cb == 0:
                        # two half reduces: banks 0-1 (first column half)
                        # reduce while banks 2-3's data is still in flight
                        nc.vector.tensor_reduce(
                            out=dst[:, 0:32], in_=pt[:, 0:32, :],
                            axis=mybir.AxisListType.X,
                            op=mybir.AluOpType.max)
                        nc.vector.tensor_reduce(
                            out=dst[:, 32:64], in_=pt[:, 32:64, :],
                            axis=mybir.AxisListType.X,
                            op=mybir.AluOpType.max)
                    else:
                        nc.vector.tensor_reduce(
                            out=dst, in_=pt[:, :nw_cb, :],
                            axis=mybir.AxisListType.X,
                            op=mybir.AluOpType.max)
                if cb == CB_SPLIT - 1:
                    for qb in range(QB):
                        nc.sync.dma_start(wm_dram[qb][:, 0:W_SPLIT], wma[qb])
                elif cb == CB_SPLIT2 - 1:
                    for qb in range(QB):
                        nc.sync.dma_start(wm_dram[qb][:, W_SPLIT:W_SPLIT2],
                                          wmb[qb])

            for qb in range(QB):
                nc.sync.dma_start(wm_dram[qb][:, W_SPLIT2:NWIN], wmc[qb])
    nc.compile()
    return nc


def _get_nc():
    global _NC_CACHE
    if _NC_CACHE is None:
        _NC_CACHE = build_nc()
    return _NC_CACHE


def _prep_in_maps(queries, f):
    qT = np.ascontiguousarray(queries.T)          # [512, 256]
    qT_f8 = np.ascontiguousarray(
        qT.reshape(KC, P, NQ).transpose(1, 0, 2)
    ).astype(ml_dtypes.float8_e4m3)               # [128, 4, 256]

    norms = np.sqrt(np.einsum('nd,nd->n', f, f, dtype=np.float64))
    scale = (FSCALE / np.maximum(norms, EPS)).astype(np.float32)

    in_maps = []
    for c in range(NCORES):
        blk = f[c * N_SHARD:(c + 1) * N_SHARD] \
            * scale[c * N_SHARD:(c + 1) * N_SHARD, None]
        fT = np.zeros((D, N_PAD), dtype=ml_dtypes.float8_e4m3)
        fT[:, :N_SHARD] = blk.T.astype(ml_dtypes.float8_e4m3)
        in_maps.append({"qT": qT_f8, "fT": fT})
    return in_maps


def run_device(queries, f, trace=False):
    """SPMD device pass.

    Returns (wval, wstart, valid, results_obj) with shapes
    [NCORES, NQ, NWIN]: device window-max values / global window start
    columns / validity mask."""
    in_maps = _prep_in_maps(queries, f)
    nc = _get_nc()
    res = bass_utils.run_bass_kernel_spmd(
        nc, in_maps, core_ids=list(range(NCORES)), trace=trace)
    wval = np.stack([np.asarray(res.results[c]["wmax"],
                                dtype=np.float32).reshape(NQ, NWIN)
                     for c in range(NCORES)])
    lstart = np.arange(NWIN) * WIN                         # local col start
    valid = (lstart < N_SHARD)[None, None, :] & np.isfinite(wval)
    wstart = lstart[None, None, :] \
        + (np.arange(NCORES) * N_SHARD)[:, None, None]
    wstart = np.where(valid, wstart, 0)
    return wval, wstart, valid, res


def knn_from_candidates(queries, f, labels, wval, wstart, valid):
    nq = queries.shape[0]
    flat_val = np.where(valid, wval, -np.inf).transpose(1, 0, 2).reshape(nq, -1)
    flat_start = wstart.transpose(1, 0, 2).reshape(nq, -1)

    t = min(T_WINDOWS, flat_val.shape[1])
    part = np.argpartition(-flat_val, t - 1, axis=1)[:, :t]
    sel_start = np.take_along_axis(flat_start, part, axis=1)   # [nq, t]

    cols = sel_start[:, :, None] + np.arange(WIN)[None, None, :]
    core_of = sel_start // N_SHARD
    col_ok = cols < (core_of[:, :, None] + 1) * N_SHARD
    cols = np.where(col_ok, cols, 0)
    flat = cols.reshape(nq, -1)                                # [nq, t*32]
    flat_ok = col_ok.reshape(nq, -1)

    # stage 1: fp32 prescore of all candidate columns (blocked); used only
    # to pick the top-R for the exact fp64 rescore, with ample margin
    qn = queries.astype(np.float64)
    qn /= np.maximum(np.linalg.norm(qn, axis=1, keepdims=True), EPS)
    qn32 = qn.astype(np.float32)
    norms64 = np.sqrt(np.einsum('nd,nd->n', f, f, dtype=np.float64))
    norms64 = np.maximum(norms64, EPS)
    inv32 = (1.0 / norms64).astype(np.float32)
    R = 3 * K
    pre_idx = np.empty((nq, R), dtype=np.int64)
    BQ = 32
    for q0 in range(0, nq, BQ):
        q1 = min(q0 + BQ, nq)
        fl = flat[q0:q1]
        rows = f[fl.reshape(-1)].reshape(q1 - q0, -1, D)
        s32 = np.einsum('qtd,qd->qt', rows, qn32[q0:q1],
                        dtype=np.float32) * inv32[fl]
        s32 = np.where(flat_ok[q0:q1], s32, -np.inf)
        pre = np.argpartition(-s32, R - 1, axis=1)[:, :R]
        pre_idx[q0:q1] = np.take_along_axis(fl, pre, axis=1)

    # stage 2: exact fp64 rescore of the survivors; top-32 with the
    # reference tie-break (lower row index wins), dropping duplicate columns
    rows = f[pre_idx.reshape(-1)].astype(np.float64).reshape(nq, R, D)
    sims = np.einsum('qtd,qd->qt', rows, qn) / norms64[pre_idx]

    by_idx = np.argsort(pre_idx, axis=1, kind='stable')
    sims_s = np.take_along_axis(sims, by_idx, axis=1)
    cidx_s = np.take_along_axis(pre_idx, by_idx, axis=1)
    dup = np.zeros_like(sims_s, dtype=bool)
    dup[:, 1:] = cidx_s[:, 1:] == cidx_s[:, :-1]
    sims_s = np.where(dup, -np.inf, sims_s)
    order = np.argsort(-sims_s, axis=1, kind='stable')[:, :K]
    top_sims = np.take_along_axis(sims_s, order, axis=1)
    top_idx = np.take_along_axis(cidx_s, order, axis=1)

    w = np.exp(top_sims.astype(np.float32) / np.float32(TEMP))
    lab = labels[top_idx]                                   # [nq, K]
    out = np.zeros((nq, NUM_CLASSES), dtype=np.float32)
    np.add.at(out, (np.arange(nq)[:, None], lab), w)
    return out


def kernel(queries, train_features, train_labels):
    queries = np.asarray(queries, dtype=np.float32)
    f = np.asarray(train_features, dtype=np.float32)
    labels = np.asarray(train_labels)
    wval, wstart, valid, _ = run_device(queries, f)
    return knn_from_candidates(queries, f, labels, wval, wstart, valid)


# revision 47
# speedup vs baseline: 1.2120x; 1.2120x over previous
"""KNN retrieval kernel for Trainium2 (8 NeuronCores, SPMD).

Pipeline (v3 — fp8 DoubleRow matmul + device window-max, host top-k):
  host:   L2-normalize datastore rows, scale x16, quantize to fp8 e4m3,
          shard N=500000 -> 8 x 62500, transpose to [512, N_loc] per core.
          Queries quantized to fp8 e4m3 (per-query scale is irrelevant to
          per-query ranking).
  device: sims = qT.T @ fT in fp8 DoubleRow (2 fp8 MACs/cell, fp32 PSUM).
          Per 512-col PSUM bank: window-max over 32-col windows
          (vector.tensor_reduce, one fused pass per 4-bank chunk) -> fp16
          SBUF.  That is the whole device reduction; the window-max array
          [2, 128, 1984] per core (1 MB) streams out in two DMA halves.
  host:   per query: take top-96 windows by device value across all cores
          (8*1984 = 15872 candidates), exactly rescore all 32 columns of
          each (fp32 prescore + fp64 refine, true L2 normalization), exact
          top-32 with the reference tie-break, W = exp(sim/T), one-hot
          label aggregation.

Recall: a true top-32 neighbor is missed only if its 32-col window is not
in the query's global top-96 windows by device value (measured max true-
item window rank 52 under fp8 quantization noise of ~2.5e-3 cosine units).
In-window collisions are harmless: the host rescores whole windows.
"""

import numpy as np
import ml_dtypes

import concourse.bass as bass
import concourse.mybir as mybir
from concourse import bacc
from concourse.tile import TileContext
from concourse import bass_utils

P = 128
D = 512            # feature dim (4 K-subtiles of 128; DoubleRow pairs 2)
KC = D // P        # 4
NQ = 256           # queries (2 partition blocks)
QB = NQ // P       # 2
NCORES = 8
N_TOTAL = 500000
N_SHARD = N_TOTAL // NCORES   # 62500
CB = 2048                      # DMA column block (one 4-bank PSUM tile and
                               # one fused window-max pass per query block)
NCOLB = (N_SHARD + CB - 1) // CB   # 31
N_PAD = CB * NCOLB                 # 63488
BANK = 512                         # one PSUM bank of fp32
WIN = 32                           # window-max width
NW_BANK = BANK // WIN              # 16
NWIN = (N_PAD - BANK) // WIN       # 1968: the last chunk's 4th bank is
                                   # pure padding and is never computed

K = 32
TEMP = 0.07
NUM_CLASSES = 1000
EPS = 1e-12
FSCALE = 16.0      # normalized features scaled into fp8's sweet spot
T_WINDOWS = 96     # windows exactly rescored per query

_NC_CACHE = None


def build_nc():
    """Single-core Bass program (run SPMD on all 8 cores)."""
    nc = bacc.Bacc("TRN2", target_bir_lowering=False, debug=False)
    q_dram = nc.dram_tensor("qT", [P, KC, NQ], mybir.dt.float8e4,
                            kind="ExternalInput").ap()
    f_dram = nc.dram_tensor("fT", [D, N_PAD], mybir.dt.float8e4,
                            kind="ExternalInput").ap()
    wm_dram = nc.dram_tensor("wmax", [QB, P, NWIN], mybir.dt.float16,
                             kind="ExternalOutput").ap()

    DR = mybir.MatmulPerfMode.DoubleRow

    with TileContext(nc) as tc:
        with (
            tc.tile_pool(name="qpool", bufs=1) as qpool,
            tc.tile_pool(name="fpool", bufs=6) as fpool,
            tc.tile_pool(name="cpool", bufs=1) as cpool,
            tc.tile_pool(name="psum", bufs=1, space="PSUM") as psum_pool,
        ):
            qt = qpool.tile([P, KC, NQ], mybir.dt.float8e4)
            nc.sync.dma_start(qt, q_dram)

            # PE warm-up: dummy matmuls on the query tile while the first
            # feature chunk is still in flight, so the tensor engine exits
            # its slow p-state before real work begins.  Results land in a
            # PSUM bank that the first real start=True matmul resets.
            wp = psum_pool.tile([P, CB // WIN, WIN], mybir.dt.float32,
                                name="pt0", tag="pt0")
            for _ in range(10):
                nc.tensor.matmul(wp[:, 0:4, :],
                                 lhsT=qt[:, 0:2, 0:P],
                                 rhs=qt[:, 2:4, 0:P],
                                 start=True, stop=True, perf_mode=DR)

            # window-max buffers split three ways: the first two stream out
            # to HBM mid-kernel so only the last two chunks' windows (112)
            # remain for the final DMA after the last reduce
            CB_SPLIT = 20
            CB_SPLIT2 = 29
            W_SPLIT = CB_SPLIT * (CB // WIN)            # 1280
            W_SPLIT2 = CB_SPLIT2 * (CB // WIN)          # 1856
            wma = [cpool.tile([P, W_SPLIT], mybir.dt.float16,
                              name=f"wma{qb}") for qb in range(QB)]
            wmb = [cpool.tile([P, W_SPLIT2 - W_SPLIT], mybir.dt.float16,
                              name=f"wmb{qb}") for qb in range(QB)]
            wmc = [cpool.tile([P, NWIN - W_SPLIT2], mybir.dt.float16,
                              name=f"wmc{qb}") for qb in range(QB)]

            for cb in range(NCOLB):
                ft = fpool.tile([P, KC, CB], mybir.dt.float8e4, tag="ft")
                if cb == 0:
                    # split the first chunk's loads into column halves so
                    # they fan out across more DMA queues and the first
                    # matmul can start sooner
                    for k in range(KC):
                        for h in range(2):
                            hw = CB // 2
                            nc.sync.dma_start(
                                ft[:, k, h * hw:(h + 1) * hw],
                                f_dram[k * P:(k + 1) * P,
                                       h * hw:(h + 1) * hw])
                else:
                    for k in range(KC):
                        nc.sync.dma_start(
                            ft[:, k, :],
                            f_dram[k * P:(k + 1) * P, cb * CB:(cb + 1) * CB])

                # the last chunk's 4th bank (cols >= 62976) is all padding
                nbanks = 3 if cb == NCOLB - 1 else CB // BANK
                for qb in range(QB):
                    # one 4-bank PSUM tile per query block; each matmul
                    # writes one bank, then a single fused window-max pass
                    # drains them all (fewer DVE instructions + handoffs)
                    pt = psum_pool.tile([P, CB // WIN, WIN],
                                        mybir.dt.float32,
                                        name=f"pt{qb}", tag=f"pt{qb}")
                    for nn in range(nbanks):
                        for p in range(2):
                            nc.tensor.matmul(
                                pt[:, nn * NW_BANK:(nn + 1) * NW_BANK, :],
                                lhsT=qt[:, 2 * p:2 * p + 2,
                                        qb * P:(qb + 1) * P],
                                rhs=ft[:, 2 * p:2 * p + 2,
                                       nn * BANK:(nn + 1) * BANK],
                                start=(p == 0), stop=(p == 1),
                                perf_mode=DR)
                    nw = CB // WIN
                    nw_cb = nbanks * NW_BANK
                    if cb < CB_SPLIT:
                        dst = wma[qb][:, cb * nw:cb * nw + nw_cb]
                    elif cb < CB_SPLIT2:
                        dst = wmb[qb][:, (cb - CB_SPLIT) * nw:
                                      (cb - CB_SPLIT) * nw + nw_cb]
                    else:
                        dst = wmc[qb][:, (cb - CB_SPLIT2) * nw:
                                      (cb - CB_SPLIT2) * nw + nw_cb]
                    nc.vector.tensor_reduce(
                        out=dst, in_=pt[:, :nw_cb, :],
                        axis=mybir.AxisListType.X,
                        op=mybir.AluOpType.max)
                if cb == CB_SPLIT - 1:
                    for qb in range(QB):
                        nc.sync.dma_start(wm_dram[qb][:, 0:W_SPLIT], wma[qb])
                elif cb == CB_SPLIT2 - 1:
                    for qb in range(QB):
                        nc.sync.dma_start(wm_dram[qb][:, W_SPLIT:W_SPLIT2],
                                          wmb[qb])

            for qb in range(QB):
                nc.sync.dma_start(wm_dram[qb][:, W_SPLIT2:NWIN], wmc[qb])
    nc.compile()
    return nc


def _get_nc():
    global _NC_CACHE
    if _NC_CACHE is None:
        _NC_CACHE = build_nc()
    return _NC_CACHE


def _prep_in_maps(queries, f):
    qT = np.ascontiguousarray(queries.T)          # [512, 256]
    qT_f8 = np.ascontiguousarray(
        qT.reshape(KC, P, NQ).transpose(1, 0, 2)
    ).astype(ml_dtypes.float8_e4m3)               # [128, 4, 256]

    norms = np.sqrt(np.einsum('nd,nd->n', f, f, dtype=np.float64))
    scale = (FSCALE / np.maximum(norms, EPS)).astype(np.float32)

    in_maps = []
    for c in range(NCORES):
        blk = f[c * N_SHARD:(c + 1) * N_SHARD] \
            * scale[c * N_SHARD:(c + 1) * N_SHARD, None]
        fT = np.zeros((D, N_PAD), dtype=ml_dtypes.float8_e4m3)
        fT[:, :N_SHARD] = blk.T.astype(ml_dtypes.float8_e4m3)
        in_maps.append({"qT": qT_f8, "fT": fT})
    return in_maps


def run_device(queries, f, trace=False):
    """SPMD device pass.

    Returns (wval, wstart, valid, results_obj) with shapes
    [NCORES, NQ, NWIN]: device window-max values / global window start
    columns / validity mask."""
    in_maps = _prep_in_maps(queries, f)
    nc = _get_nc()
    res = bass_utils.run_bass_kernel_spmd(
        nc, in_maps, core_ids=list(range(NCORES)), trace=trace)
    wval = np.stack([np.asarray(res.results[c]["wmax"],
                                dtype=np.float32).reshape(NQ, NWIN)
                     for c in range(NCORES)])
    lstart = np.arange(NWIN) * WIN                         # local col start
    valid = (lstart < N_SHARD)[None, None, :] & np.isfinite(wval)
    wstart = lstart[None, None, :] \
        + (np.arange(NCORES) * N_SHARD)[:, None, None]
    wstart = np.where(valid, wstart, 0)
    return wval, wstart, valid, res


def knn_from_candidates(queries, f, labels, wval, wstart, valid):
    nq = queries.shape[0]
    flat_val = np.where(valid, wval, -np.inf).transpose(1, 0, 2).reshape(nq, -1)
    flat_start = wstart.transpose(1, 0, 2).reshape(nq, -1)

    t = min(T_WINDOWS, flat_val.shape[1])
    part = np.argpartition(-flat_val, t - 1, axis=1)[:, :t]
    sel_start = np.take_along_axis(flat_start, part, axis=1)   # [nq, t]

    cols = sel_start[:, :, None] + np.arange(WIN)[None, None, :]
    core_of = sel_start // N_SHARD
    col_ok = cols < (core_of[:, :, None] + 1) * N_SHARD
    cols = np.where(col_ok, cols, 0)
    flat = cols.reshape(nq, -1)                                # [nq, t*32]
    flat_ok = col_ok.reshape(nq, -1)

    # stage 1: fp32 prescore of all candidate columns (blocked); used only
    # to pick the top-R for the exact fp64 rescore, with ample margin
    qn = queries.astype(np.float64)
    qn /= np.maximum(np.linalg.norm(qn, axis=1, keepdims=True), EPS)
    qn32 = qn.astype(np.float32)
    norms64 = np.sqrt(np.einsum('nd,nd->n', f, f, dtype=np.float64))
    norms64 = np.maximum(norms64, EPS)
    inv32 = (1.0 / norms64).astype(np.float32)
    R = 3 * K
    pre_idx = np.empty((nq, R), dtype=np.int64)
    BQ = 32
    for q0 in range(0, nq, BQ):
        q1 = min(q0 + BQ, nq)
        fl = flat[q0:q1]
        rows = f[fl.reshape(-1)].reshape(q1 - q0, -1, D)
        s32 = np.einsum('qtd,qd->qt', rows, qn32[q0:q1],
                        dtype=np.float32) * inv32[fl]
        s32 = np.where(flat_ok[q0:q1], s32, -np.inf)
        pre = np.argpartition(-s32, R - 1, axis=1)[:, :R]
        pre_idx[q0:q1] = np.take_along_axis(fl, pre, axis=1)

    # stage 2: exact fp64 rescore of the survivors; top-32 with the
    # reference tie-break (lower row index wins), dropping duplicate columns
    rows = f[pre_idx.reshape(-1)].astype(np.float64).reshape(nq, R, D)
    sims = np.einsum('qtd,qd->qt', rows, qn) / norms64[pre_idx]

    by_idx = np.argsort(pre_idx, axis=1, kind='stable')
    sims_s = np.take_along_axis(sims, by_idx, axis=1)
    cidx_s = np.take_along_axis(pre_idx, by_idx, axis=1)
    dup = np.zeros_like(sims_s, dtype=bool)
    dup[:, 1:] = cidx_s[:, 1:] == cidx_s[:, :-1]
    sims_s = np.where(dup, -np.inf, sims_s)
    order = np.argsort(-sims_s, axis=1, kind='stable')[:, :K]
    top_sims = np.take_along_axis(sims_s, order, axis=1)
    top_idx = np.take_along_axis(cidx_s, order, axis=1)

    w = np.exp(top_sims.astype(np.float32) / np.float32(TEMP))
    lab = labels[top_idx]                                   # [nq, K]
    out = np.zeros((nq, NUM_CLASSES), dtype=np.float32)
    np.add.at(out, (np.arange(nq)[:, None], lab), w)
    return out


def kernel(queries, train_features, train_labels):
    queries = np.asarray(queries, dtype=np.float32)
    f = np.asarray(train_features, dtype=np.float32)
    labels = np.asarray(train_labels)
    wval, wstart, valid, _ = run_device(queries, f)
    return knn_from_candidates(queries, f, labels, wval, wstart, valid)
